# revision 5
# baseline (speedup 1.0000x reference)
"""Trainium2 Bass kernel for ColorFlowLayer GNN message passing.

Strategy (8 NeuronCores, SPMD) — the axon tunnel moves ~80 MB/s one way,
so the design minimizes host<->device bytes above all else:
  - Shard EDGES by destination-node range: core c owns global nodes
    [c*6272, (c+1)*6272) and every edge whose dst falls there, so the
    per-node segment-sum needs no collective.
  - Algebraic split of edge-MLP layer 1 (linear before silu):
        z1_e = A[src_e] + B[dst_e] + R[rel_e]
    with per-node tables A = h@W1[:,src half] + RA[role] + CA[col],
    B likewise for the dst half, R = rel_emb@W1_rel + eb1.
  - A and B are computed ON DEVICE from the sharded h (3 matmuls per
    128-node window); A is AllGather'd across the 8 cores into a full
    [50176, 128] f32 table; per-edge rows are then fetched with gpsimd
    indirect DMAs (128 rows per instruction).  Host->device traffic is
    the fp16 h shard plus narrow-int edge indices (~2.8 MB/core instead
    of the ~49 MB/core a host-side f32 pre-gather needs).
  - Edges are sorted by dst on host and padded into 128-edge tiles that
    never span a 128-node window; the segment-sum is a chain of PE
    matmuls against a one-hot (edge->node) matrix accumulated in PSUM.
  - fp16 on the wire (h in, big weights in), f32 compute on device;
    fp16 keeps 8x more mantissa than bf16 for these bounded values at
    the same byte cost.  The output returns as per-row int8 with one
    f32 scale per node (max|row|/126), halving the slower (~50 MB/s)
    device->host direction; dequantized on host.
  - The jitted PJRT executable is built once and cached; repeat calls
    pipeline: the h upload is enqueued first and streams while the edge
    sort runs on the host, donated output buffers are pre-created on
    device during the previous call's fetch.
Sync rules learned the hard way: no manual then_inc/wait_ge semaphores
around DMAs/collectives inside TileContext (device deadlock / collective
sync-slot overflow); a strict_bb_all_engine_barrier before the
collective plus the Tile scheduler's automatic edges is sufficient.
"""

import numpy as np

H = 128
P = 128
NCORES = 8
NS = 6272           # nodes per core = 49 windows * 128
NW = NS // P        # 49
NPAD = NCORES * NS  # 50176
BLK = 8             # edge tiles per compute block (1024 edges)
TW_FIXED = 16       # tiles per 128-node window (2048 edge slots) if it fits
LN_EPS = 1e-5

_CACHE = {}
_LAST_EXEC_NS = None


# --------------------------------------------------------------------------
# host-side preprocessing (pure numpy, vectorized)
# --------------------------------------------------------------------------

def _meta_quick(dst, eb2, nb2, ln_g, ln_b):
    """Tile schedule + kernel-variant flags; cheap (no sort needed)."""
    NGW = NCORES * NW
    cnt = np.bincount(dst // P, minlength=NGW)
    cnts = cnt.reshape(NCORES, NW)
    maxw = int(cnts.max())
    if maxw <= TW_FIXED * P:
        T = np.full(NW, TW_FIXED, np.int64)
    else:
        T = np.maximum(1, -(-cnts.max(axis=0) // P)).astype(np.int64)
        padt = (-int(T.sum())) % BLK
        T[NW - 1] += padt
    NT = int(T.sum())
    meta = dict(
        NT=NT, T=tuple(int(t) for t in T),
        has_eb2=bool(np.any(np.asarray(eb2) != 0)),
        has_nb2=bool(np.any(np.asarray(nb2) != 0)),
        ln_id=bool(np.all(np.asarray(ln_g) == 1)
                   and np.all(np.asarray(ln_b) == 0)),
    )
    return meta, cnt


def _prep_stage1(h, node_color_rep, node_role, rel_emb, role_emb, color_emb,
                 eW1, eb1, eb2, nW1, nb1, nW2, eW2, ln_g, ln_b, nb2,
                 h_cat=None):
    """Node-table inputs: weights, h shard, one-hots (no edge data)."""
    f32 = np.float32
    N = np.asarray(h).shape[0]
    role = np.asarray(node_role).astype(np.int64)
    col = np.asarray(node_color_rep).astype(np.int64)

    eW1 = np.asarray(eW1, f32)
    W1_hs = np.ascontiguousarray(eW1[0:128])
    W1_hd = np.ascontiguousarray(eW1[128:256])
    Rtab = (np.asarray(rel_emb, f32) @ eW1[256:272]
            + np.asarray(eb1, f32))                               # [8,128]
    RA = np.asarray(role_emb, f32) @ eW1[272:280]                 # [6,128]
    RB = np.asarray(role_emb, f32) @ eW1[280:288]
    CA = np.asarray(color_emb, f32) @ eW1[288:296]                # [3,128]
    CB = np.asarray(color_emb, f32) @ eW1[296:304]
    nW1 = np.asarray(nW1, f32)
    nW1_h = np.ascontiguousarray(nW1[0:128])
    nW1_agg = np.ascontiguousarray(nW1[128:256])
    NRtab = (np.asarray(role_emb, f32) @ nW1[256:264]
             + np.asarray(nb1, f32))                              # [6,128]
    NCtab = np.asarray(color_emb, f32) @ nW1[264:272]

    nodes = np.arange(N)
    ncore = nodes // NS
    nloc = nodes % NS
    rolehot = np.zeros((NCORES, 6, NS), np.int8)
    rolehot[ncore, role, nloc] = 1
    colhot = np.zeros((NCORES, 3, NS), np.int8)
    colhot[ncore, col, nloc] = 1
    if h_cat is None:
        h_cat = np.zeros((NPAD, H), np.float16)
        h_cat[:N] = np.asarray(h, f32).astype(np.float16)

    def rep(x):
        return np.tile(np.asarray(x, np.float32), (NCORES, 1))

    def rep16(x):
        return np.tile(np.asarray(x, np.float32).astype(np.float16),
                       (NCORES, 1))

    return {
        "h_mine": h_cat,
        "rolehot": rolehot.reshape(NCORES * 6, NS),
        "colhot": colhot.reshape(NCORES * 3, NS),
        "W1_hs": rep16(W1_hs), "W1_hd": rep16(W1_hd),
        "Rtab": rep(Rtab), "RA": rep(RA), "RB": rep(RB),
        "CA": rep(CA), "CB": rep(CB),
        "eW2": rep16(eW2), "nW1_h": rep16(nW1_h), "nW1_agg": rep16(nW1_agg),
        "NRtab": rep(NRtab), "NCtab": rep(NCtab), "nW2": rep16(nW2),
        "eb2row": rep(np.asarray(eb2, np.float32).reshape(1, H)),
        "nb2row": rep(np.asarray(nb2, np.float32).reshape(1, H)),
        "lng": rep(np.asarray(ln_g, np.float32).reshape(1, H)),
        "lnb": rep(np.asarray(ln_b, np.float32).reshape(1, H)),
    }


def _prep_stage2(src, dst, rel, meta, cnt):
    """Edge slot arrays (needs the sort)."""
    f32 = np.float32
    E = src.shape[0]
    NT = meta["NT"]
    T = np.asarray(meta["T"], np.int64)
    offs = np.zeros(NW + 1, np.int64)
    offs[1:] = np.cumsum(T)

    order = np.argsort(dst, kind="stable")
    srcs = src[order]
    dsts = dst[order]
    rels = rel[order]
    gw = dsts // P                       # global window id, sorted
    NGW = NCORES * NW
    starts = np.zeros(NGW, np.int64)
    starts[1:] = np.cumsum(cnt)[:-1]
    rank = np.arange(E, dtype=np.int64) - starts[gw]
    wloc = gw % NW
    core = gw // NW
    slot = (offs[wloc] + rank // P) * P + rank % P
    flat = core * (NT * P) + slot

    SZ = NCORES * NT * P
    srcv = np.zeros(SZ, np.uint16)
    srcv[flat] = srcs
    dstloc = np.zeros(SZ, np.int16)
    dstloc[flat] = dsts % NS
    dstwv = np.full(SZ, -1, np.int8)
    dstwv[flat] = dsts % P
    relv = np.zeros(SZ, np.int8)
    relv[flat] = rels

    def tileize(a):
        # [8, NT, P] -> per-core column-major tile layout [8*P, NT]
        return np.ascontiguousarray(
            a.reshape(NCORES, NT, P).transpose(0, 2, 1)
        ).reshape(NCORES * P, NT)

    return {
        "srcv": tileize(srcv),
        "dstloc": tileize(dstloc),
        "dstwv": tileize(dstwv),
        "relv": tileize(relv),
    }


def _prep_host(h, edge_index, edge_relation, node_color_rep, node_role,
               rel_emb, role_emb, color_emb,
               eW1, eb1, eW2, eb2, nW1, nb1, nW2, nb2, ln_g, ln_b):
    src = np.asarray(edge_index[0]).astype(np.int64)
    dst = np.asarray(edge_index[1]).astype(np.int64)
    rel = np.asarray(edge_relation).astype(np.int64)
    meta, cnt = _meta_quick(dst, eb2, nb2, ln_g, ln_b)
    in_map = _prep_stage1(h, node_color_rep, node_role, rel_emb, role_emb,
                          color_emb, eW1, eb1, eb2, nW1, nb1, nW2, eW2,
                          ln_g, ln_b, nb2)
    in_map.update(_prep_stage2(src, dst, rel, meta, cnt))
    return in_map, meta, np.asarray(h).shape[0]


# --------------------------------------------------------------------------
# device kernel
# --------------------------------------------------------------------------

def _build_nc(meta):
    import concourse.bass as bass
    import concourse.bacc as bacc
    import concourse.mybir as mybir
    import concourse.tile as tile
    from concourse.masks import make_identity
    from contextlib import ExitStack

    NT = meta["NT"]
    T = meta["T"]
    AF = mybir.ActivationFunctionType
    dt = mybir.dt
    nc = bacc.Bacc()

    def inp(name, shape, dty=dt.float32):
        return nc.dram_tensor(name, shape, dty, kind="ExternalInput")

    h_mine = inp("h_mine", [NS, H], dt.float16)
    srcv_d = inp("srcv", [P, NT], dt.uint16)
    dstloc_d = inp("dstloc", [P, NT], dt.int16)
    dstwv_d = inp("dstwv", [P, NT], dt.int8)
    relv_d = inp("relv", [P, NT], dt.int8)
    rolehot_d = inp("rolehot", [6, NS], dt.int8)
    colhot_d = inp("colhot", [3, NS], dt.int8)
    W1_hs_d = inp("W1_hs", [H, H], dt.float16)
    W1_hd_d = inp("W1_hd", [H, H], dt.float16)
    Rtab_d = inp("Rtab", [8, H])
    RA_d = inp("RA", [6, H]); RB_d = inp("RB", [6, H])
    CA_d = inp("CA", [3, H]); CB_d = inp("CB", [3, H])
    eW2_d = inp("eW2", [H, H], dt.float16)
    nW1_h_d = inp("nW1_h", [H, H], dt.float16)
    nW1_agg_d = inp("nW1_agg", [H, H], dt.float16)
    nW2_d = inp("nW2", [H, H], dt.float16)
    NR_d = inp("NRtab", [6, H]); NC_d = inp("NCtab", [3, H])
    eb2_d = inp("eb2row", [1, H]); nb2_d = inp("nb2row", [1, H])
    lng_d = inp("lng", [1, H]); lnb_d = inp("lnb", [1, H])

    out_q = nc.dram_tensor("out_q", [NS, H], dt.int8, kind="ExternalOutput")
    out_s = nc.dram_tensor("out_s", [NS, 1], dt.float32,
                           kind="ExternalOutput")

    A_shard = nc.dram_tensor("A_shard", [NS, H], dt.float32)
    B_loc = nc.dram_tensor("B_loc", [NS, H], dt.float32)
    A_all = nc.dram_tensor("A_all", [NPAD, H], dt.float32,
                           addr_space="Shared")

    with tile.TileContext(nc) as tc, ExitStack() as ctx:
        cst = ctx.enter_context(tc.tile_pool(name="cst", bufs=1))
        big = ctx.enter_context(tc.tile_pool(name="big", bufs=1))

        ident = cst.tile([P, P], dt.float32)
        make_identity(nc, ident[:])
        identB = cst.tile([P, P], dt.float16)
        nc.vector.tensor_copy(out=identB[:], in_=ident[:])
        W1_hs = cst.tile([H, H], dt.float32)
        W1_hd = cst.tile([H, H], dt.float32)
        RA = cst.tile([6, H], dt.float32); RB = cst.tile([6, H], dt.float32)
        CA = cst.tile([3, H], dt.float32); CB = cst.tile([3, H], dt.float32)
        eW2 = cst.tile([H, H], dt.float32)
        nW1_h = cst.tile([H, H], dt.float32)
        nW1_agg = cst.tile([H, H], dt.float32)
        NRt = cst.tile([6, H], dt.float32); NCt = cst.tile([3, H], dt.float32)
        nW2 = cst.tile([H, H], dt.float32)
        w16 = {}
        for nm in ("W1_hs", "W1_hd", "eW2", "nW1_h", "nW1_agg", "nW2"):
            w16[nm] = cst.tile([H, H], dt.float16, name=f"w16_{nm}")
        eb2r = cst.tile([1, H], dt.float32); nb2r = cst.tile([1, H], dt.float32)
        ones1 = cst.tile([1, P], dt.float32)
        lng = cst.tile([1, H], dt.float32); lnb = cst.tile([1, H], dt.float32)
        lngP = cst.tile([P, H], dt.float32)
        lnbP = cst.tile([P, H], dt.float32)
        for t, d in [(RA, RA_d), (RB, RB_d), (CA, CA_d), (CB, CB_d),
                     (NRt, NR_d), (NCt, NC_d),
                     (eb2r, eb2_d), (nb2r, nb2_d), (lng, lng_d), (lnb, lnb_d)]:
            nc.sync.dma_start(t[:], d[:])
        for t16, t32, d in [(w16["W1_hs"], W1_hs, W1_hs_d),
                            (w16["W1_hd"], W1_hd, W1_hd_d),
                            (w16["eW2"], eW2, eW2_d),
                            (w16["nW1_h"], nW1_h, nW1_h_d),
                            (w16["nW1_agg"], nW1_agg, nW1_agg_d),
                            (w16["nW2"], nW2, nW2_d)]:
            nc.sync.dma_start(t16[:], d[:])
            nc.vector.tensor_copy(out=t32[:], in_=t16[:])
        nc.vector.memset(ones1[:], 1.0)

        srcv_n = big.tile([P, NT], dt.uint16)
        srcv = big.tile([P, NT], dt.int32)
        dstloc_n = big.tile([P, NT], dt.int16)
        dstloc = big.tile([P, NT], dt.int32)
        dstwv = big.tile([P, NT], dt.int8)
        relv_n = big.tile([P, NT], dt.int8)
        relv = big.tile([P, NT], dt.int32)
        rolehot_n = big.tile([6, NS], dt.int8)
        rolehot = big.tile([6, NS], dt.float32)
        colhot_n = big.tile([3, NS], dt.int8)
        colhot = big.tile([3, NS], dt.float32)
        h_raw = big.tile([P, NW, H], dt.float16)   # [node, w, feat]
        hT = big.tile([P, NW, H], dt.float32)      # [feat, w, node]
        nc.sync.dma_start(srcv_n[:], srcv_d[:])
        nc.sync.dma_start(dstloc_n[:], dstloc_d[:])
        nc.sync.dma_start(dstwv[:], dstwv_d[:])
        nc.sync.dma_start(relv_n[:], relv_d[:])
        nc.sync.dma_start(rolehot_n[:], rolehot_d[:])
        nc.sync.dma_start(colhot_n[:], colhot_d[:])
        nc.vector.tensor_copy(out=srcv[:], in_=srcv_n[:])
        nc.vector.tensor_copy(out=rolehot[:], in_=rolehot_n[:])
        nc.vector.tensor_copy(out=colhot[:], in_=colhot_n[:])
        nc.vector.tensor_copy(out=dstloc[:], in_=dstloc_n[:])
        nc.vector.tensor_copy(out=relv[:], in_=relv_n[:])
        # DVE-owned copies: the one-hot is_equal (a 3D-broadcast
        # TensorTensor) only has room for one sync wait in its ISA
        # encoding, so both its inputs must come from same-engine (DVE)
        # producers instead of DMA-written tiles.
        dstwv_w = big.tile([P, NT], dt.float32)
        iota_i = big.tile([P, BLK, P], dt.int32)
        iota_w = big.tile([P, BLK, P], dt.float32)
        nc.vector.tensor_copy(out=dstwv_w[:], in_=dstwv[:])
        nc.gpsimd.iota(iota_i[:], pattern=[[0, BLK], [1, P]],
                       base=0, channel_multiplier=0)
        nc.vector.tensor_copy(out=iota_w[:], in_=iota_i[:])

        # ---------------- phase 0: hT, A_shard, B_loc ----------------
        with tc.tile_pool(name="p0s", bufs=4) as p0s, \
             tc.tile_pool(name="p0p", bufs=2, space="PSUM") as p0p, \
             tc.tile_pool(name="p0q", bufs=2, space="PSUM") as p0q:
            if not meta["ln_id"]:
                # broadcast ln scale/shift rows to all 128 partitions via PE
                pg = p0q.tile([P, H], dt.float32, tag="pa")
                nc.tensor.matmul(out=pg[:], lhsT=ones1[:], rhs=lng[:],
                                 start=True, stop=True)
                nc.vector.tensor_copy(out=lngP[:], in_=pg[:])
                pgb = p0q.tile([P, H], dt.float32, tag="pb")
                nc.tensor.matmul(out=pgb[:], lhsT=ones1[:], rhs=lnb[:],
                                 start=True, stop=True)
                nc.vector.tensor_copy(out=lnbP[:], in_=pgb[:])
            for w in range(NW):
                sl = slice(w * P, (w + 1) * P)
                nc.sync.dma_start(h_raw[:, w, :], h_mine[sl, :])
                pt = p0p.tile([P, P], dt.float16, tag="tr")
                nc.tensor.transpose(out=pt[:], in_=h_raw[:, w, :],
                                    identity=identB[:])
                nc.vector.tensor_copy(out=hT[:, w, :], in_=pt[:])
                pa = p0q.tile([P, P], dt.float32, tag="pa")
                nc.tensor.matmul(out=pa[:], lhsT=hT[:, w, :], rhs=W1_hs[:],
                                 start=True, stop=False)
                nc.tensor.matmul(out=pa[:], lhsT=rolehot[:, sl], rhs=RA[:],
                                 start=False, stop=False)
                nc.tensor.matmul(out=pa[:], lhsT=colhot[:, sl], rhs=CA[:],
                                 start=False, stop=True)
                asb = p0s.tile([P, H], dt.float32, tag="a")
                nc.vector.tensor_copy(out=asb[:], in_=pa[:])
                nc.sync.dma_start(A_shard[sl, :], asb[:])
                pb = p0q.tile([P, P], dt.float32, tag="pb")
                nc.tensor.matmul(out=pb[:], lhsT=hT[:, w, :], rhs=W1_hd[:],
                                 start=True, stop=False)
                nc.tensor.matmul(out=pb[:], lhsT=rolehot[:, sl], rhs=RB[:],
                                 start=False, stop=False)
                nc.tensor.matmul(out=pb[:], lhsT=colhot[:, sl], rhs=CB[:],
                                 start=False, stop=True)
                bsb = p0s.tile([P, H], dt.float32, tag="b")
                nc.vector.tensor_copy(out=bsb[:], in_=pb[:])
                nc.sync.dma_start(B_loc[sl, :], bsb[:])

        # The strict barrier's backward edges wait on the A_shard/B_loc DMA
        # completion semaphores; the Tile scheduler auto-inserts the
        # collective->reader sync edge (both validated in isolation).
        # Manual then_inc/wait_ge semaphores here deadlock the device.
        tc.strict_bb_all_engine_barrier()
        nc.gpsimd.collective_compute(
            "AllGather", mybir.AluOpType.bypass,
            replica_groups=[list(range(NCORES))],
            ins=[A_shard[:].opt()],
            outs=[A_all[:].opt()],
        )

        # ---------------- edge + node phases ----------------
        w_first = {}
        w_last = {}
        t2w = []
        for w in range(NW):
            for _ in range(T[w]):
                t2w.append(w)
        for t, w in enumerate(t2w):
            w_first.setdefault(w, t)
            w_last[w] = t

        with tc.tile_pool(name="gta", bufs=2) as gta, \
             tc.tile_pool(name="gtb", bufs=2) as gtb, \
             tc.tile_pool(name="gtr", bufs=2) as gtr, \
             tc.tile_pool(name="zsb", bufs=2) as zsbp, \
             tc.tile_pool(name="ohp", bufs=2) as ohp, \
             tc.tile_pool(name="y1p", bufs=2) as y1p, \
             tc.tile_pool(name="msb", bufs=3) as msb, \
             tc.tile_pool(name="nod", bufs=2) as nod, \
             tc.tile_pool(name="zps", bufs=2, space="PSUM") as zps, \
             tc.tile_pool(name="mps", bufs=2, space="PSUM") as mps, \
             tc.tile_pool(name="aps", bufs=1, space="PSUM") as aps, \
             tc.tile_pool(name="nps", bufs=1, space="PSUM") as nps:

            agg_ps = None
            for t0 in range(0, NT, BLK):
                Ag = gta.tile([P, BLK, H], dt.float32, tag="ag")
                Bg = gtb.tile([P, BLK, H], dt.float32, tag="bg")
                Rg = gtr.tile([P, BLK, H], dt.float32, tag="rg")
                for s in range(BLK):
                    t = t0 + s
                    nc.gpsimd.indirect_dma_start(
                        out=Ag[:, s, :], out_offset=None,
                        in_=A_all[:],
                        in_offset=bass.IndirectOffsetOnAxis(
                            ap=srcv[:, t:t + 1], axis=0))
                    nc.gpsimd.indirect_dma_start(
                        out=Bg[:, s, :], out_offset=None,
                        in_=B_loc[:],
                        in_offset=bass.IndirectOffsetOnAxis(
                            ap=dstloc[:, t:t + 1], axis=0))
                    nc.gpsimd.indirect_dma_start(
                        out=Rg[:, s, :], out_offset=None,
                        in_=Rtab_d[:],
                        in_offset=bass.IndirectOffsetOnAxis(
                            ap=relv[:, t:t + 1], axis=0))
                zsum = zsbp.tile([P, BLK, H], dt.float32, tag="z")
                nc.vector.tensor_add(out=zsum[:], in0=Ag[:], in1=Bg[:])
                nc.vector.tensor_add(out=zsum[:], in0=zsum[:], in1=Rg[:])

                oh = ohp.tile([P, BLK, P], dt.float32, tag="oh")
                nc.vector.tensor_tensor(
                    out=oh[:],
                    in0=dstwv_w[:, t0:t0 + BLK].unsqueeze(2).to_broadcast(
                        [P, BLK, P]),
                    in1=iota_w[:],
                    op=mybir.AluOpType.is_equal)

                zp = zps.tile([P, BLK * P], dt.float32, tag="z")
                for s in range(BLK):
                    nc.tensor.matmul(out=zp[:, s * P:(s + 1) * P],
                                     lhsT=zsum[:, s, :], rhs=ident[:],
                                     start=True, stop=True,
                                     is_transpose=True)
                y1 = y1p.tile([P, BLK * P], dt.float32, tag="y1")
                nc.scalar.activation(y1[:], zp[:], AF.Silu)

                for half in range(2):
                    mp = mps.tile([P, 4 * P], dt.float32, tag="m")
                    for s4 in range(4):
                        s = half * 4 + s4
                        nc.tensor.matmul(out=mp[:, s4 * P:(s4 + 1) * P],
                                         lhsT=y1[:, s * P:(s + 1) * P],
                                         rhs=eW2[:],
                                         start=True, stop=not meta["has_eb2"])
                        if meta["has_eb2"]:
                            nc.tensor.matmul(out=mp[:, s4 * P:(s4 + 1) * P],
                                             lhsT=ones1[:],
                                             rhs=eb2r[:], start=False,
                                             stop=True)
                    ms = msb.tile([P, 4 * P], dt.float32, tag="ms")
                    nc.scalar.activation(ms[:], mp[:], AF.Silu)
                    for s4 in range(4):
                        s = half * 4 + s4
                        t = t0 + s
                        w = t2w[t]
                        if t == w_first[w]:
                            agg_ps = aps.tile([P, P], dt.float32, tag="agg")
                        nc.tensor.matmul(out=agg_ps[:],
                                         lhsT=ms[:, s4 * P:(s4 + 1) * P],
                                         rhs=oh[:, s, :],
                                         start=(t == w_first[w]),
                                         stop=(t == w_last[w]))
                        if t == w_last[w]:
                            # ---------- node phase for window w ----------
                            aggT = nod.tile([P, P], dt.float32, tag="aggT")
                            nc.vector.tensor_copy(out=aggT[:], in_=agg_ps[:])
                            zn = nps.tile([P, P], dt.float32, tag="n")
                            nc.tensor.matmul(out=zn[:], lhsT=nW1_h[:],
                                             rhs=hT[:, w, :],
                                             start=True, stop=False)
                            nc.tensor.matmul(out=zn[:], lhsT=nW1_agg[:],
                                             rhs=aggT[:],
                                             start=False, stop=False)
                            nc.tensor.matmul(out=zn[:], lhsT=NRt[:],
                                             rhs=rolehot[:, w * P:(w + 1) * P],
                                             start=False, stop=False)
                            nc.tensor.matmul(out=zn[:], lhsT=NCt[:],
                                             rhs=colhot[:, w * P:(w + 1) * P],
                                             start=False, stop=True)
                            y1n = nod.tile([P, P], dt.float32, tag="y1n")
                            nc.scalar.activation(y1n[:], zn[:], AF.Silu)
                            up = nps.tile([P, P], dt.float32, tag="n")
                            nc.tensor.matmul(out=up[:], lhsT=y1n[:],
                                             rhs=nW2[:],
                                             start=True,
                                             stop=not meta["has_nb2"])
                            if meta["has_nb2"]:
                                nc.tensor.matmul(out=up[:], lhsT=ones1[:],
                                                 rhs=nb2r[:], start=False,
                                                 stop=True)
                            hw = nod.tile([P, H], dt.float32, tag="hw")
                            nc.vector.tensor_copy(out=hw[:],
                                                  in_=h_raw[:, w, :])
                            x = nod.tile([P, H], dt.float32, tag="x")
                            nc.vector.tensor_add(out=x[:], in0=up[:],
                                                 in1=hw[:])
                            # layernorm along free axis
                            mu = nod.tile([P, 1], dt.float32, tag="mu")
                            nc.vector.reduce_sum(out=mu[:], in_=x[:],
                                                 axis=mybir.AxisListType.X)
                            nc.vector.tensor_scalar_mul(mu[:], mu[:],
                                                        -1.0 / H)
                            xc = nod.tile([P, H], dt.float32, tag="xc")
                            nc.vector.tensor_scalar_add(xc[:], x[:], mu[:])
                            sq = nod.tile([P, H], dt.float32, tag="sq")
                            nc.vector.tensor_mul(out=sq[:], in0=xc[:],
                                                 in1=xc[:])
                            var = nod.tile([P, 1], dt.float32, tag="var")
                            nc.vector.reduce_sum(out=var[:], in_=sq[:],
                                                 axis=mybir.AxisListType.X)
                            nc.vector.tensor_scalar(
                                out=var[:], in0=var[:],
                                scalar1=1.0 / H, scalar2=LN_EPS,
                                op0=mybir.AluOpType.mult,
                                op1=mybir.AluOpType.add)
                            std = nod.tile([P, 1], dt.float32, tag="std")
                            nc.scalar.activation(std[:], var[:], AF.Sqrt)
                            rstd = nod.tile([P, 1], dt.float32, tag="rstd")
                            nc.vector.reciprocal(out=rstd[:], in_=std[:])
                            o = nod.tile([P, H], dt.float32, tag="o")
                            nc.vector.tensor_scalar_mul(o[:], xc[:], rstd[:])
                            if not meta["ln_id"]:
                                nc.vector.tensor_mul(out=o[:], in0=o[:],
                                                     in1=lngP[:])
                                nc.vector.tensor_add(out=o[:], in0=o[:],
                                                     in1=lnbP[:])
                            # per-row int8 quantization: s = max|o|/126,
                            # oq = o/s  (126 leaves headroom below the int8
                            # saturation boundary)
                            ab = nod.tile([P, H], dt.float32, tag="ab")
                            nc.scalar.activation(ab[:], o[:], AF.Abs)
                            rm = nod.tile([P, 1], dt.float32, tag="rm")
                            nc.vector.reduce_max(out=rm[:], in_=ab[:],
                                                 axis=mybir.AxisListType.X)
                            qs = nod.tile([P, 1], dt.float32, tag="qs")
                            nc.vector.tensor_scalar(
                                out=qs[:], in0=rm[:],
                                scalar1=1e-30, scalar2=1.0 / 126.0,
                                op0=mybir.AluOpType.max,
                                op1=mybir.AluOpType.mult)
                            iv = nod.tile([P, 1], dt.float32, tag="iv")
                            nc.vector.reciprocal(out=iv[:], in_=qs[:])
                            oq = nod.tile([P, H], dt.int8, tag="oq")
                            nc.vector.tensor_scalar_mul(oq[:], o[:], iv[:])
                            nc.sync.dma_start(out_q[w * P:(w + 1) * P, :],
                                              oq[:])
                            nc.sync.dma_start(out_s[w * P:(w + 1) * P, :],
                                              qs[:])
    nc.finalize()
    return nc


# --------------------------------------------------------------------------
# cached PJRT runner (built once per compiled nc, reused across calls)
# --------------------------------------------------------------------------

def _make_runner(nc):
    import jax
    import jax.numpy as jnp
    from jax.sharding import Mesh, PartitionSpec, NamedSharding
    from jax.experimental.shard_map import shard_map
    import concourse.mybir as mybir
    from concourse.bass2jax import (install_neuronx_cc_hook, _bass_exec_p,
                                    partition_id_tensor)

    install_neuronx_cc_hook()

    partition_name = (nc.partition_id_tensor.name
                      if nc.partition_id_tensor else None)
    in_names, out_names, out_avals = [], [], []
    for alloc in nc.m.functions[0].allocations:
        if not isinstance(alloc, mybir.MemoryLocationSet):
            continue
        name = alloc.memorylocations[0].name
        if alloc.kind == "ExternalInput":
            if name != partition_name:
                in_names.append(name)
        elif alloc.kind == "ExternalOutput":
            shape = tuple(alloc.tensor_shape)
            dtype = mybir.dt.np(alloc.dtype)
            out_names.append(name)
            out_avals.append(jax.core.ShapedArray(shape, dtype))
    n_params = len(in_names)
    n_outs = len(out_names)
    all_names = list(in_names) + list(out_names)
    if partition_name is not None:
        all_names.append(partition_name)
    all_names = tuple(all_names)
    donate = tuple(range(n_params, n_params + n_outs))

    def _body(*args):
        operands = list(args)
        if partition_name is not None:
            operands.append(partition_id_tensor())
        outs = _bass_exec_p.bind(
            *operands,
            out_avals=tuple(out_avals),
            in_names=all_names,
            out_names=tuple(out_names),
            lowering_input_output_aliases=(),
            sim_require_finite=True,
            sim_require_nnan=True,
            nc=nc,
        )
        return tuple(outs)

    devices = jax.devices()[:NCORES]
    mesh = Mesh(np.asarray(devices), ("core",))
    in_specs = (PartitionSpec("core"),) * (n_params + n_outs)
    out_specs = (PartitionSpec("core"),) * n_outs
    sharded = jax.jit(
        shard_map(_body, mesh=mesh, in_specs=in_specs, out_specs=out_specs,
                  check_rep=False),
        donate_argnums=donate, keep_unused=True)
    zshard = NamedSharding(mesh, PartitionSpec("core"))

    def _zeros():
        return tuple(
            jnp.zeros((NCORES * a.shape[0], *a.shape[1:]), a.dtype)
            for a in out_avals)

    zfn = jax.jit(_zeros, out_shardings=(zshard,) * n_outs)

    return dict(sharded=sharded, zfn=zfn, in_names=in_names,
                out_names=out_names, zshard=zshard)


def kernel(**inputs):
    import time
    import os
    import jax
    global _LAST_EXEC_NS
    t0 = time.time()
    _prof = bool(os.environ.get("KPROF"))

    def _tick(label):
        if _prof:
            print(f"  [kprof] {label:24s} +{(time.time()-t0)*1e3:8.1f} ms",
                  flush=True)

    src = np.asarray(inputs["edge_index"][0]).astype(np.int64)
    dst = np.asarray(inputs["edge_index"][1]).astype(np.int64)
    rel = np.asarray(inputs["edge_relation"]).astype(np.int64)
    meta, cnt = _meta_quick(dst, inputs["eb2"], inputs["nb2"],
                            inputs["ln_g"], inputs["ln_b"])
    key = (meta["NT"], meta["T"], meta["has_eb2"], meta["has_nb2"],
           meta["ln_id"])
    if key not in _CACHE:
        nc = _build_nc(meta)
        _CACHE[key] = _make_runner(nc)
    R = _CACHE[key]
    sh = R["zshard"]

    # enqueue the big h upload first so it streams over the wire while the
    # remaining host prep (weights, one-hots, edge sort) runs
    _tick("meta+cache")
    h_in = np.asarray(inputs["h"])
    N_in = h_in.shape[0]
    h_cat = np.zeros((NPAD, H), np.float16)
    np.copyto(h_cat[:N_in], h_in, casting="unsafe")
    _tick("h quant")
    put_h = jax.device_put(h_cat, sh)
    _tick("h put enq")

    s1 = _prep_stage1(inputs["h"], inputs["node_color_rep"],
                      inputs["node_role"], inputs["rel_emb"],
                      inputs["role_emb"], inputs["color_emb"],
                      inputs["eW1"], inputs["eb1"], inputs["eb2"],
                      inputs["nW1"], inputs["nb1"], inputs["nW2"],
                      inputs["eW2"], inputs["ln_g"], inputs["ln_b"],
                      inputs["nb2"], h_cat=h_cat)
    del s1["h_mine"]
    _tick("stage1 prep")
    vals1 = list(s1.values())
    put1 = dict(zip(s1.keys(), jax.device_put(vals1, [sh] * len(vals1))))
    put1["h_mine"] = put_h
    _tick("stage1 put enq")
    zeros = R.pop("zeros_next", None) or R["zfn"]()

    # stage 2 (edge slotting) overlaps with the stage-1 transfer
    s2 = _prep_stage2(src, dst, rel, meta, cnt)
    _tick("stage2 prep")
    vals2 = list(s2.values())
    put2 = dict(zip(s2.keys(), jax.device_put(vals2, [sh] * len(vals2))))
    _tick("stage2 put enq")

    dev = {**put1, **put2}
    args = [dev[n] for n in R["in_names"]] + list(zeros)
    if _prof:
        for a in args:
            a.block_until_ready()
        _tick("all puts done")
    outs = R["sharded"](*args)
    if _prof:
        for o in outs:
            o.block_until_ready()
        _tick("exec done")
    # prepare the next call's donated output buffers while fetching
    R["zeros_next"] = R["zfn"]()
    iq = R["out_names"].index("out_q")
    isc = R["out_names"].index("out_s")
    import threading
    sbox = {}

    def _fetch_s():
        sbox["s"] = np.asarray(outs[isc])

    th = threading.Thread(target=_fetch_s)
    th.start()
    full_q = np.asarray(outs[iq])
    th.join()
    full_s = sbox["s"]
    _tick("fetch done")
    res = full_q[:N_in].astype(np.float32) * full_s[:N_in]
    # full end-to-end wall time of this call (prep + transfer + exec + fetch)
    _LAST_EXEC_NS = int((time.time() - t0) * 1e9)
    return res

